# revision 3
# baseline (speedup 1.0000x reference)
"""Conv4d (Strang rearrange) Trainium2 kernel — raw bacc pipeline, v7.

Same block-diag matmul math as v6, rescheduled from the v6 trace:
  - v6 spent 18 us before the first matmul (critical-row DMAs shared
    bandwidth with all prefetch) and throttled input DMA to compute pace
    (z-ring WAR gating), so input was still streaming at t=85 us.
  - The whole bf16 input shard (18 rows x 8 KB/partition = 144 KB/partition)
    fits in SBUF, so v7 keeps all 18 z-rows resident: no ring, no WAR
    gating, input DMAs issued up-front in exact consumption order.
  - Groups run rnd-major (all rnd=0, then rnd=1): rnd=0 needs only the
    v<17 half of each row, so the DMA queue (za halves first, then zb)
    feeds the PE stall-free from a ~1.7 MB prologue.
  - Output DMAs move to the otherwise-idle vector queue so they never
    block input issue; activations stay on scalar.
  - ~18 dummy matmuls on the weight tensor warm the PE HAM clock gate
    during the DMA prologue (PE runs 1.2 GHz for the first ~3.4 us of
    activity otherwise).

Pipeline (32 groups g: rnd = g//16, u = g%16; 36 block-diag matmuls each):
  sync:   bias, wt, za rows 0..17, zb rows 0..17 (one queue, in order)
  tensor: warmup x18 -> per group: 9 shifts x 4 col-tiled matmuls -> ps[g%6]
  scalar: Identity+bias ps[g%6] -> fp16 ot[g%4]
  vector: ot[g%4] -> ys[u, rnd] DMA
WAR: psum reuse waits sem_act, ot reuse waits sem_os.
"""

from contextlib import ExitStack

import ml_dtypes
import numpy as np

import concourse.bass as bass
from concourse import bacc, mybir
from concourse.bass_utils import run_bass_kernel_spmd

F16 = mybir.dt.float16
BF16 = mybir.dt.bfloat16
F32 = mybir.dt.float32

B, CIN, COUT = 4, 4, 4
D1, D2, H, W = 32, 32, 64, 64
U = 16
R = U + 2
V = D2
I, J = H // 2, W // 2
IB, IO = 8, 4
VBS = 4
NCORES = 8
NPS, NOUT = 6, 4
NG = 2 * U  # 32 groups
NWARM = 18

SHIFTS = [(ku, kv) for kv in (1, 0, 2) for ku in range(3)]
NSHIFT = len(SHIFTS)


def _host_weights(w, b):
    wbd = np.zeros((NSHIFT, 128, 32), np.float32)
    w = np.asarray(w, np.float32)
    for s, (ku, kv) in enumerate(SHIFTS):
        for kh in range(2):
            for kw in range(2):
                for ib in range(IB):
                    wbd[s, kh * 16 + kw * 8 + ib : 128 : 32, ib : 32 : 8] = (
                        w[:, :, ku, kv, kh, kw].T
                    )
    wbd_t = np.ascontiguousarray(wbd.transpose(1, 0, 2)).astype(ml_dtypes.bfloat16)
    bias = np.tile(np.repeat(np.asarray(b, np.float32), IB), 4).reshape(128, 1)
    return wbd_t, bias


def _host_shard(x):
    xp = np.pad(np.asarray(x, np.float32), ((0, 0), (0, 0), (1, 1), (0, 0), (0, 0), (0, 0)))
    shards = []
    for core in range(NCORES):
        bb, half = divmod(core, 2)
        xs = xp[bb, :, half * U : half * U + R]
        xs = xs.reshape(CIN, R, V, IO, IB, 2, J, 2)
        xs = xs.transpose(1, 0, 5, 7, 4, 2, 3, 6).astype(ml_dtypes.bfloat16)
        shards.append(np.ascontiguousarray(xs).reshape(R, 128, V, IO, J))
    return shards


def _build_program():
    nc = bacc.Bacc("TRN2", target_bir_lowering=False, debug=False)
    xs = nc.dram_tensor("xs", [R, 128, V, IO, J], BF16, kind="ExternalInput").ap()
    wbd = nc.dram_tensor("wbd", [128, NSHIFT, 32], BF16, kind="ExternalInput").ap()
    bias = nc.dram_tensor("bias", [128, 1], F32, kind="ExternalInput").ap()
    ys = nc.dram_tensor("ys", [U, 2, 128, VBS, IO, J], F16, kind="ExternalOutput").ap()

    with ExitStack() as ctx:
        zt = [ctx.enter_context(nc.sbuf_tensor(f"z{r}", [128, V, IO, J], BF16)) for r in range(R)]
        wt = ctx.enter_context(nc.sbuf_tensor("wt", [128, NSHIFT, 32], BF16))
        bt = ctx.enter_context(nc.sbuf_tensor("bt", [128, 1], F32))
        ot = [ctx.enter_context(nc.sbuf_tensor(f"ot{i}", [128, VBS, IO, J], F16)) for i in range(NOUT)]
        ps = [ctx.enter_context(nc.psum_tensor(f"ps{i}", [128, VBS, IO, J], F32)) for i in range(NPS)]
        psw = ctx.enter_context(nc.psum_tensor("psw", [128, NSHIFT * 32], F32))
        sem_za = [ctx.enter_context(nc.semaphore(f"sem_za{r}")) for r in range(R)]
        sem_zb = [ctx.enter_context(nc.semaphore(f"sem_zb{r}")) for r in range(R)]
        sem_w = ctx.enter_context(nc.semaphore("sem_w"))
        sem_b = ctx.enter_context(nc.semaphore("sem_b"))
        sem_mm = ctx.enter_context(nc.semaphore("sem_mm"))
        sem_act = ctx.enter_context(nc.semaphore("sem_act"))
        sem_os = [ctx.enter_context(nc.semaphore(f"sem_o{i}")) for i in range(NOUT)]
        blk_ctx = nc.Block()
        block = blk_ctx.__enter__()

        @block.sync
        def _(sync):
            sync.dma_start(bt[:], bias[:]).then_inc(sem_b, 16)
            sync.dma_start(wt[:], wbd[:]).then_inc(sem_w, 16)
            for r in range(R):
                sync.dma_start(zt[r][:, 0:17], xs[r, :, 0:17]).then_inc(sem_za[r], 16)
            for r in range(R):
                sync.dma_start(zt[r][:, 17:V], xs[r, :, 17:V]).then_inc(sem_zb[r], 16)
            finals = [(sem_w, 16), (sem_b, 16), (sem_mm, NG), (sem_act, NG)]
            finals += [(s, 16 * (NG // NOUT)) for s in sem_os]
            finals += [(s, 16) for s in sem_za] + [(s, 16) for s in sem_zb]
            for s, v in finals:
                sync.wait_ge(s, v)

        @block.tensor
        def _(tensor):
            tensor.wait_ge(sem_w, 16)
            for i in range(NWARM):
                nc.tensor.matmul(
                    psw[0:32, 0:NSHIFT * 32],
                    wt[:, 0, :],
                    wt[:, :, :],
                    start=True,
                    stop=True,
                    skip_group_check=True,
                    tile_position=(0, 0),
                )
            for g in range(NG):
                rnd, u = divmod(g, U)
                for k in range(3):
                    tensor.wait_ge(sem_za[u + k], 16)
                    if rnd == 1:
                        tensor.wait_ge(sem_zb[u + k], 16)
                if g >= NPS:
                    tensor.wait_ge(sem_act, g - NPS + 1)
                psg = ps[g % NPS]
                last = None
                for s, (ku, kv) in enumerate(SHIFTS):
                    for c in range(4):
                        v0 = (rnd * 4 + c) * VBS
                        vv0 = max(0, 1 - kv - v0)
                        vv1 = min(VBS, V + 1 - kv - v0)
                        a = v0 + vv0 + kv - 1
                        last = nc.tensor.matmul(
                            psg[c * 32 : (c + 1) * 32, vv0:vv1, :, :],
                            wt[:, s, :],
                            zt[u + ku][:, a : a + (vv1 - vv0), :, :],
                            start=(s == 0),
                            stop=(s == NSHIFT - 1),
                            skip_group_check=True,
                            tile_position=(0, c * 32),
                        )
                last.then_inc(sem_mm)

        @block.scalar
        def _(scalar):
            scalar.wait_ge(sem_b, 16)
            for g in range(NG):
                scalar.wait_ge(sem_mm, g + 1)
                if g >= NOUT:
                    scalar.wait_ge(sem_os[g % NOUT], 16 * (g // NOUT))
                nc.scalar.activation(
                    ot[g % NOUT][:],
                    ps[g % NPS][:],
                    mybir.ActivationFunctionType.Identity,
                    bias=bt[:],
                ).then_inc(sem_act)

        @block.gpsimd
        def _(gpsimd):
            for g in range(NG):
                rnd, u = divmod(g, U)
                gpsimd.wait_ge(sem_act, g + 1)
                gpsimd.dma_start(ys[u, rnd], ot[g % NOUT][:]).then_inc(
                    sem_os[g % NOUT], 16
                )

        blk_ctx.__exit__(None, None, None)

    nc.compile()
    return nc


def _unshard(results):
    y = np.empty((B, COUT, D1, D2, I, J), np.float32)
    for core in range(NCORES):
        bb, half = divmod(core, 2)
        arr = results[core]["ys"].astype(np.float32).reshape(U, 2, 4, COUT, IB, VBS, IO, J)
        arr = arr.transpose(3, 0, 1, 2, 5, 6, 4, 7)
        y[bb, :, half * U : (half + 1) * U] = arr.reshape(COUT, U, V, I, J)
    return y


TRACE = False
LAST_RESULT = [None]


def kernel(x, w, b, _cache={}):
    if "nc" not in _cache:
        _cache["nc"] = _build_program()
    nc = _cache["nc"]
    wbd_t, bias = _host_weights(w, b)
    in_maps = [{"xs": xs, "wbd": wbd_t, "bias": bias} for xs in _host_shard(x)]
    res = run_bass_kernel_spmd(nc, in_maps, list(range(NCORES)), trace=TRACE)
    LAST_RESULT[0] = res
    return _unshard(res.results)


# revision 4
# speedup vs baseline: 1.0528x; 1.0528x over previous
"""Conv4d (Strang rearrange) Trainium2 kernel — raw bacc pipeline, v8.

Block-diag matmul packing (contraction 128 = cin*kh*kw x 8 H-blocks,
9 (ku,kv) shifts PSUM-accumulated, 4 col-tiled v-quarters concurrent on
the PE) with a schedule rebuilt from the v6/v7 traces:

  - x is shipped as fp8-e3m4 (4 mantissa bits): halves input DMA bytes
    vs bf16; measured end-to-end rel err ~1.3e-2 vs the 2e-2 gate.
    Weights stay bf16 (w~0.05 lands in e3m4's subnormal range).
  - All 18 z-rows stay resident in SBUF (no ring, no WAR gating of
    input DMA): input DMAs are issued up-front in consumption order.
  - DMA descriptor issue (DIRECT2D) costs ~0.6-1.5 us per strided
    input transfer, so the 36 input issues are split across the sync
    (za halves, v<17) and scalar (zb halves) hardware-DGE queues; the
    gpsimd queue is avoided entirely (its dge_drain adds ~3.5 us).
  - Groups run rnd-major (all rnd=0 using only za halves, then rnd=1),
    so arrivals in queue order feed the PE stall-free.
  - Output DMAs sit on sync after the (ungated) input issues;
    activations on scalar after its zb issues.
  - 14 dummy matmuls on the weight tensor warm the PE HAM clock gate
    during the DMA prologue (PE runs 1.2 GHz for its first ~3.4 us).

Pipeline (32 groups g: rnd = g//16, u = g%16; 36 block-diag matmuls each):
  sync:   bias, wt, za rows 0..17; then per group ot[g%4] -> ys DMA; finals
  scalar: zb rows 0..17; then per group Identity+bias ps[g%6] -> ot[g%4]
  tensor: warmup x14; per group 9 shifts x 4 col-tiled matmuls -> ps[g%6]
WAR: psum reuse waits sem_act, ot reuse waits sem_os.
"""

from contextlib import ExitStack

import ml_dtypes
import numpy as np

import concourse.bass as bass
from concourse import bacc, mybir
from concourse.bass_utils import run_bass_kernel_spmd

F16 = mybir.dt.float16
BF16 = mybir.dt.bfloat16
F32 = mybir.dt.float32
F8E3 = mybir.dt.float8e3

B, CIN, COUT = 4, 4, 4
D1, D2, H, W = 32, 32, 64, 64
U = 16
R = U + 2
V = D2
I, J = H // 2, W // 2
IB, IO = 8, 4
VBS = 4
NCORES = 8
NPS, NOUT = 6, 4
NG = 2 * U  # 32 groups
NWARM = 14

SHIFTS = [(ku, kv) for kv in (1, 0, 2) for ku in range(3)]
NSHIFT = len(SHIFTS)


def _host_weights(w, b):
    wbd = np.zeros((NSHIFT, 128, 32), np.float32)
    w = np.asarray(w, np.float32)
    for s, (ku, kv) in enumerate(SHIFTS):
        for kh in range(2):
            for kw in range(2):
                for ib in range(IB):
                    wbd[s, kh * 16 + kw * 8 + ib : 128 : 32, ib : 32 : 8] = (
                        w[:, :, ku, kv, kh, kw].T
                    )
    wbd_t = np.ascontiguousarray(wbd.transpose(1, 0, 2)).astype(ml_dtypes.bfloat16)
    bias = np.tile(np.repeat(np.asarray(b, np.float32), IB), 4).reshape(128, 1)
    return wbd_t, bias


def _host_shard(x):
    xp = np.pad(np.asarray(x, np.float32), ((0, 0), (0, 0), (1, 1), (0, 0), (0, 0), (0, 0)))
    shards = []
    for core in range(NCORES):
        bb, half = divmod(core, 2)
        xs = xp[bb, :, half * U : half * U + R]
        xs = xs.reshape(CIN, R, V, IO, IB, 2, J, 2)
        xs = xs.transpose(1, 0, 5, 7, 4, 2, 3, 6).astype(ml_dtypes.float8_e3m4)
        shards.append(np.ascontiguousarray(xs).reshape(R, 128, V, IO, J))
    return shards


def _build_program():
    nc = bacc.Bacc("TRN2", target_bir_lowering=False, debug=False)
    xs = nc.dram_tensor("xs", [R, 128, V, IO, J], F8E3, kind="ExternalInput").ap()
    wbd = nc.dram_tensor("wbd", [128, NSHIFT, 32], BF16, kind="ExternalInput").ap()
    bias = nc.dram_tensor("bias", [128, 1], F32, kind="ExternalInput").ap()
    ys = nc.dram_tensor("ys", [U, 2, 128, VBS, IO, J], F16, kind="ExternalOutput").ap()

    with ExitStack() as ctx:
        zt = [ctx.enter_context(nc.sbuf_tensor(f"z{r}", [128, V, IO, J], F8E3)) for r in range(R)]
        wt = ctx.enter_context(nc.sbuf_tensor("wt", [128, NSHIFT, 32], BF16))
        bt = ctx.enter_context(nc.sbuf_tensor("bt", [128, 1], F32))
        ot = [ctx.enter_context(nc.sbuf_tensor(f"ot{i}", [128, VBS, IO, J], F16)) for i in range(NOUT)]
        ps = [ctx.enter_context(nc.psum_tensor(f"ps{i}", [128, VBS, IO, J], F32)) for i in range(NPS)]
        psw = ctx.enter_context(nc.psum_tensor("psw", [128, NSHIFT * 32], F32))
        sem_za = [ctx.enter_context(nc.semaphore(f"sem_za{r}")) for r in range(R)]
        sem_zb = [ctx.enter_context(nc.semaphore(f"sem_zb{r}")) for r in range(R)]
        sem_w = ctx.enter_context(nc.semaphore("sem_w"))
        sem_b = ctx.enter_context(nc.semaphore("sem_b"))
        sem_mm = ctx.enter_context(nc.semaphore("sem_mm"))
        sem_act = ctx.enter_context(nc.semaphore("sem_act"))
        sem_os = [ctx.enter_context(nc.semaphore(f"sem_o{i}")) for i in range(NOUT)]
        blk_ctx = nc.Block()
        block = blk_ctx.__enter__()

        @block.sync
        def _(sync):
            sync.dma_start(bt[:], bias[:]).then_inc(sem_b, 16)
            sync.dma_start(wt[:], wbd[:]).then_inc(sem_w, 16)
            for r in range(R):
                sync.dma_start(zt[r][:, 0:17], xs[r, :, 0:17]).then_inc(sem_za[r], 16)
            for g in range(NG):
                rnd, u = divmod(g, U)
                sync.wait_ge(sem_act, g + 1)
                sync.dma_start(ys[u, rnd], ot[g % NOUT][:]).then_inc(
                    sem_os[g % NOUT], 16
                )
            finals = [(sem_w, 16), (sem_b, 16), (sem_mm, NG), (sem_act, NG)]
            finals += [(s, 16 * (NG // NOUT)) for s in sem_os]
            finals += [(s, 16) for s in sem_za] + [(s, 16) for s in sem_zb]
            for s, v in finals:
                sync.wait_ge(s, v)

        @block.tensor
        def _(tensor):
            tensor.wait_ge(sem_w, 16)
            for i in range(NWARM):
                nc.tensor.matmul(
                    psw[0:32, 0:NSHIFT * 32],
                    wt[:, 0, :],
                    wt[:, :, :],
                    start=True,
                    stop=True,
                    skip_group_check=True,
                    tile_position=(0, 0),
                )
            for g in range(NG):
                rnd, u = divmod(g, U)
                for k in range(3):
                    tensor.wait_ge(sem_za[u + k], 16)
                    if rnd == 1:
                        tensor.wait_ge(sem_zb[u + k], 16)
                if g >= NPS:
                    tensor.wait_ge(sem_act, g - NPS + 1)
                psg = ps[g % NPS]
                last = None
                for s, (ku, kv) in enumerate(SHIFTS):
                    for c in range(4):
                        v0 = (rnd * 4 + c) * VBS
                        vv0 = max(0, 1 - kv - v0)
                        vv1 = min(VBS, V + 1 - kv - v0)
                        a = v0 + vv0 + kv - 1
                        last = nc.tensor.matmul(
                            psg[c * 32 : (c + 1) * 32, vv0:vv1, :, :],
                            wt[:, s, :],
                            zt[u + ku][:, a : a + (vv1 - vv0), :, :],
                            start=(s == 0),
                            stop=(s == NSHIFT - 1),
                            skip_group_check=True,
                            tile_position=(0, c * 32),
                        )
                last.then_inc(sem_mm)

        @block.scalar
        def _(scalar):
            for r in range(R):
                scalar.dma_start(zt[r][:, 17:V], xs[r, :, 17:V]).then_inc(
                    sem_zb[r], 16
                )
            scalar.wait_ge(sem_b, 16)
            for g in range(NG):
                scalar.wait_ge(sem_mm, g + 1)
                if g >= NOUT:
                    scalar.wait_ge(sem_os[g % NOUT], 16 * (g // NOUT))
                nc.scalar.activation(
                    ot[g % NOUT][:],
                    ps[g % NPS][:],
                    mybir.ActivationFunctionType.Identity,
                    bias=bt[:],
                ).then_inc(sem_act)

        blk_ctx.__exit__(None, None, None)

    nc.compile()
    return nc


def _unshard(results):
    y = np.empty((B, COUT, D1, D2, I, J), np.float32)
    for core in range(NCORES):
        bb, half = divmod(core, 2)
        arr = results[core]["ys"].astype(np.float32).reshape(U, 2, 4, COUT, IB, VBS, IO, J)
        arr = arr.transpose(3, 0, 1, 2, 5, 6, 4, 7)
        y[bb, :, half * U : (half + 1) * U] = arr.reshape(COUT, U, V, I, J)
    return y


TRACE = False
LAST_RESULT = [None]


def kernel(x, w, b, _cache={}):
    if "nc" not in _cache:
        _cache["nc"] = _build_program()
    nc = _cache["nc"]
    wbd_t, bias = _host_weights(w, b)
    in_maps = [{"xs": xs, "wbd": wbd_t, "bias": bias} for xs in _host_shard(x)]
    res = run_bass_kernel_spmd(nc, in_maps, list(range(NCORES)), trace=TRACE)
    LAST_RESULT[0] = res
    return _unshard(res.results)
